# revision 1
# baseline (speedup 1.0000x reference)
"""DeepSet2d Trainium2 kernel.

Network (reference): per-pixel MLPs over a 224x224 image treated as a set of
N=50176 (loc, rgb) tokens, softplus, sum-pool over the set, then a small
classifier MLP.

Device decomposition (per NeuronCore, tokens sharded N/8 = 6272 per core,
all 32 samples on every core):

  h1   = relu(w_obs1^T x + b_obs1)                      [128, F]  (PE K=3, row-group paired)
  h2   = relu(Wf^T h1 + Wl^T em_loc^T + bf)             [128, F]  (PE K=128 + K=64 PSUM-accum)
  z    = w_ol2^T h2                                     [64, F]   (PE K=128, col-group paired)
  acc += sum_tokens ln(1 + exp(z + b_ol2))              (ACT Exp+Ln, accum_out on Ln)

where Wf = w_obs2 @ w_ol1[:64] and bf = b_obs2 @ w_ol1[:64] + b_ol1 fold the
em_obs layer into the obs-loc layer (em_obs is linear, never materialized),
and em_loc (batch-independent, "replicated" per the sharding hint) is
precomputed host-side and streamed in as a [64, 6272] bf16 constant.

Chunks of 512 tokens are processed in pairs: even chunks sit at SBUF
partitions 0-2 (PE row-group 0), odd chunks at 32-34 (row-group 1), so the
two K=3 mm1 matmuls execute concurrently in disjoint row groups; likewise
the K=64 loc matmuls use row groups 0-1 vs 2-3, and the M=64 final matmuls
use column groups 0-1 vs 2-3. Elementwise ops run on [128, 1024] pair tiles,
split between ACT and DVE by greedy load balancing.

The device returns per-(core, sample, quad) partial channel sums; the host
adds the 8 per-core partials and applies the tiny classifier MLP (0.6 MFLOP).
"""

import numpy as np
import ml_dtypes
from contextlib import ExitStack

import concourse.bass as bass
import concourse.bacc as bacc
import concourse.tile as tile
from concourse import mybir
from concourse.bass_utils import run_bass_kernel_spmd

B, C, H, W = 32, 3, 224, 224
N = H * W                      # 50176
HID, EM, NCLS = 128, 64, 10
NCORES = 8
NTOK = N // NCORES             # 6272 tokens per core
F = 512                        # chunk (= one PSUM bank of fp32)
NFULL = NTOK // F              # 12 full chunks per sample
NPAIR = NFULL // 2             # 6 pairs
NQUAD = NPAIR // 2             # 3 quads (2 pairs each)
TAIL = NTOK - NFULL * F        # 128
XCOLS = NPAIR * F + TAIL       # 3200 columns per row-group in the packed x
# accum columns: B*NQUAD quad sums + B/2 shared tail columns (tails of
# samples (2i, 2i+1) share one column: top 64 partitions = even sample,
# bottom 64 = odd sample)
ACC_COLS = B * NQUAD + B // 2

BF16 = mybir.dt.bfloat16
F32 = mybir.dt.float32
npbf16 = ml_dtypes.bfloat16

_BUILT = None

# pool-tuning knobs (swept via simulator)
CFG = {"p1_bufs": 3, "p2_bufs": 3, "p4_bufs": 1, "s_bufs": 4, "sp_bufs": 3,
       "x_bufs": 3}


def _build_nc():
    nc = bacc.Bacc()
    AF = mybir.ActivationFunctionType
    ALU = mybir.AluOpType

    x_in = nc.declare_dram_parameter("x", [B, 2, C, XCOLS], BF16, isOutput=False)
    eml_in = nc.declare_dram_parameter("eml", [EM, NTOK], BF16, isOutput=False)
    w1_in = nc.declare_dram_parameter("w1", [35, HID], BF16, isOutput=False)
    wf_in = nc.declare_dram_parameter("wf", [HID, HID], BF16, isOutput=False)
    wl_in = nc.declare_dram_parameter("wl", [HID, HID], BF16, isOutput=False)
    w4_in = nc.declare_dram_parameter("w4", [HID, EM], BF16, isOutput=False)
    b1_in = nc.declare_dram_parameter("b1", [HID, 1], F32, isOutput=False)
    bf_in = nc.declare_dram_parameter("bf", [HID, 1], F32, isOutput=False)
    b4_in = nc.declare_dram_parameter("b4", [HID, 1], F32, isOutput=False)
    acc_out = nc.declare_dram_parameter("acc", [HID, ACC_COLS], F32, isOutput=True)

    # greedy ACT/DVE load balancing for the relu passes (ns estimates)
    eng_ns = {"act": 0.0, "dve": 0.0}

    def relu_cost(fd, eng):
        return (270 + fd) / 1.2 if eng == "act" else (120 + fd) / 0.96

    with ExitStack() as ctx:
        tc = ctx.enter_context(tile.TileContext(nc))
        consts = ctx.enter_context(tc.tile_pool(name="consts", bufs=1))
        xpool = ctx.enter_context(tc.tile_pool(name="xpool", bufs=CFG["x_bufs"]))
        s1p = ctx.enter_context(tc.tile_pool(name="s1p", bufs=CFG["s_bufs"]))
        s2p = ctx.enter_context(tc.tile_pool(name="s2p", bufs=CFG["s_bufs"]))
        spp = ctx.enter_context(tc.tile_pool(name="spp", bufs=CFG["sp_bufs"]))
        accp = ctx.enter_context(tc.tile_pool(name="accp", bufs=1))
        p1pool = ctx.enter_context(tc.tile_pool(name="p1pool", bufs=CFG["p1_bufs"], space="PSUM"))
        p2pool = ctx.enter_context(tc.tile_pool(name="p2pool", bufs=CFG["p2_bufs"], space="PSUM"))
        p4pool = ctx.enter_context(tc.tile_pool(name="p4pool", bufs=CFG["p4_bufs"], space="PSUM"))

        emlt = consts.tile([HID, NTOK], BF16)
        nc.sync.dma_start(out=emlt[0:EM, :], in_=eml_in[:, :])
        nc.sync.dma_start(out=emlt[EM:HID, :], in_=eml_in[:, :])
        w1t = consts.tile([35, HID], BF16)
        nc.sync.dma_start(out=w1t, in_=w1_in[:, :])
        wf = consts.tile([HID, HID], BF16)
        nc.sync.dma_start(out=wf, in_=wf_in[:, :])
        wlt = consts.tile([HID, HID], BF16)
        nc.sync.dma_start(out=wlt, in_=wl_in[:, :])
        w4 = consts.tile([HID, EM], BF16)
        nc.sync.dma_start(out=w4, in_=w4_in[:, :])
        b1 = consts.tile([HID, 1], F32)
        nc.sync.dma_start(out=b1, in_=b1_in[:, :])
        bf = consts.tile([HID, 1], F32)
        nc.sync.dma_start(out=bf, in_=bf_in[:, :])
        b4 = consts.tile([HID, 1], F32)
        nc.sync.dma_start(out=b4, in_=b4_in[:, :])

        acc = accp.tile([HID, ACC_COLS], F32)
        nc.vector.memset(acc, 0.0)

        def relu_bias(out_t, in_t, bias_t, fd):
            a, d = relu_cost(fd, "act"), relu_cost(fd, "dve")
            if eng_ns["act"] + a <= eng_ns["dve"] + d:
                eng_ns["act"] += a
                nc.scalar.activation(out_t, in_t, AF.Relu, bias=bias_t)
            else:
                eng_ns["dve"] += d
                nc.vector.tensor_scalar(out_t, in_t, bias_t, 0.0, ALU.add, ALU.max)

        def chunk_to_s2(xs, g, xc, c0, fd):
            """One chunk: x row-group g cols [xc, xc+fd) -> s2 tile [HID, fd].
            c0 = token offset for em_loc columns."""
            pt = p1pool.tile([HID, F], F32, tag="p1")
            nc.tensor.matmul(pt[:, 0:fd], w1t[32 * g:32 * g + C, :],
                             xs[32 * g:32 * g + C, xc:xc + fd],
                             start=True, stop=True)
            s1t = s1p.tile([HID, F], BF16, tag="s1")
            relu_bias(s1t[:, 0:fd], pt[:, 0:fd], b1, fd)
            qt = p2pool.tile([HID, F], F32, tag="p2")
            nc.tensor.matmul(qt[:, 0:fd], wf, s1t[:, 0:fd], start=True, stop=False)
            nc.tensor.matmul(qt[:, 0:fd], wlt[EM * g:EM * g + EM, :],
                             emlt[EM * g:EM * g + EM, c0:c0 + fd],
                             start=False, stop=True)
            s2t = s2p.tile([HID, F], BF16, tag="s2")
            relu_bias(s2t[:, 0:fd], qt[:, 0:fd], bf, fd)
            return s2t

        xs_prev = None
        for b in range(B):
            xs = xpool.tile([35, XCOLS], BF16, tag="xs")
            nc.sync.dma_start(out=xs[0:C, :], in_=x_in[b, 0])
            nc.sync.dma_start(out=xs[32:32 + C, :], in_=x_in[b, 1])

            for qd in range(NQUAD):
                rt = p4pool.tile([HID, 2 * F], F32, tag="p4")
                # pre-charge the quad's Exp+Ln so the relu balancer sees it
                eng_ns["act"] += (222 + 2 * F) / 1.2 + (224 + 2 * F) / 1.2
                for h in range(2):
                    p = 2 * qd + h
                    s2a = chunk_to_s2(xs, 0, p * F, 2 * p * F, F)
                    s2b = chunk_to_s2(xs, 1, p * F, 2 * p * F + F, F)
                    nc.tensor.matmul(rt[0:EM, h * F:h * F + F], w4, s2a[:, :],
                                     start=True, stop=True)
                    nc.tensor.matmul(rt[EM:HID, h * F:h * F + F], w4, s2b[:, :],
                                     start=True, stop=True)
                # softplus(z + b4) = Ln(Exp(z + b4) + 1); Exp and Ln share one
                # ACT table set; the quad's token-sum rides Ln's accum_out
                ext = spp.tile([HID, 2 * F], F32, tag="ex")
                nc.scalar.activation(ext, rt, AF.Exp, bias=b4)
                spt = spp.tile([HID, 2 * F], BF16, tag="sp")
                col = b * NQUAD + qd
                nc.scalar.activation(spt, ext, AF.Ln, bias=1.0,
                                     accum_out=acc[:, col:col + 1])

            # tail chunks (TAIL tokens each) are paired across adjacent
            # samples: even sample's tail -> partitions 0-63, odd sample's
            # -> 64-127, one Exp/Ln per pair of samples
            if b % 2 == 1:
                s2t0 = chunk_to_s2(xs_prev, 0, NPAIR * F, NFULL * F, TAIL)
                s2t1 = chunk_to_s2(xs, 0, NPAIR * F, NFULL * F, TAIL)
                rt = p4pool.tile([HID, 2 * F], F32, tag="p4")
                nc.tensor.matmul(rt[0:EM, 0:TAIL], w4, s2t0[:, 0:TAIL],
                                 start=True, stop=True)
                nc.tensor.matmul(rt[EM:HID, 0:TAIL], w4, s2t1[:, 0:TAIL],
                                 start=True, stop=True)
                ext = spp.tile([HID, 2 * F], F32, tag="ex")
                nc.scalar.activation(ext[:, 0:TAIL], rt[:, 0:TAIL], AF.Exp,
                                     bias=b4)
                spt = spp.tile([HID, 2 * F], BF16, tag="sp")
                col = B * NQUAD + b // 2
                eng_ns["act"] += (222 + TAIL) / 1.2 + (224 + TAIL) / 1.2
                nc.scalar.activation(spt[:, 0:TAIL], ext[:, 0:TAIL], AF.Ln,
                                     bias=1.0, accum_out=acc[:, col:col + 1])
            xs_prev = xs

        nc.sync.dma_start(out=acc_out[:, :], in_=acc)

    # All ACT funcs used here (Relu/Exp/Ln) live in the single table set
    # natural_log_exp_and_others. The table-load inserter maps each func to
    # the FIRST set containing it, which alternates sets (Relu->exp_and_others,
    # Ln->natural_log) and emits a ~2.7us table reload per transition. Strip
    # our funcs from every other set (dict order, and thus set ids, preserved)
    # so everything resolves to the one shared set -> exactly one load.
    AF = mybir.ActivationFunctionType
    import concourse.bacc as _bacc_mod
    _orig_tables = _bacc_mod.get_activation_tables
    _mine = {AF.Relu, AF.Exp, AF.Ln}
    _keep = "natural_log_exp_and_others"

    def _patched_tables(arch):
        t = _orig_tables(arch)
        assert _keep in t and _mine <= t[_keep], (list(t), t.get(_keep))
        return {n: (s if n == _keep else s - _mine) for n, s in t.items()}

    _bacc_mod.get_activation_tables = _patched_tables
    try:
        nc.compile()
    finally:
        _bacc_mod.get_activation_tables = _orig_tables
    return nc


def _get_built():
    global _BUILT
    if _BUILT is None:
        _BUILT = _build_nc()
    return _BUILT


def _pack_x(x_core):
    """[96, 6272] f32 -> [B, 2, 3, XCOLS] bf16: per sample, even chunks
    (+tail) in row-group 0, odd chunks in row-group 1."""
    out = np.zeros((B, 2, C, XCOLS), npbf16)
    for b in range(B):
        xb = x_core[3 * b:3 * b + 3]                    # [3, 6272]
        full = xb[:, :NFULL * F].reshape(C, NFULL, F)
        even = full[:, 0::2].reshape(C, NPAIR * F)
        odd = full[:, 1::2].reshape(C, NPAIR * F)
        out[b, 0, :, :NPAIR * F] = even.astype(npbf16)
        out[b, 0, :, NPAIR * F:] = xb[:, NFULL * F:].astype(npbf16)
        out[b, 1, :, :NPAIR * F] = odd.astype(npbf16)
    return out


def kernel(images, w_obs1, b_obs1, w_obs2, b_obs2,
           w_loc1, b_loc1, w_loc2, b_loc2,
           w_ol1, b_ol1, w_ol2, b_ol2,
           w_cls1, b_cls1, w_cls2, b_cls2):
    images = np.asarray(images, np.float32)
    f32 = lambda a: np.asarray(a, np.float32)
    w_obs1, b_obs1, w_obs2, b_obs2 = map(f32, (w_obs1, b_obs1, w_obs2, b_obs2))
    w_loc1, b_loc1, w_loc2, b_loc2 = map(f32, (w_loc1, b_loc1, w_loc2, b_loc2))
    w_ol1, b_ol1, w_ol2, b_ol2 = map(f32, (w_ol1, b_ol1, w_ol2, b_ol2))
    w_cls1, b_cls1, w_cls2, b_cls2 = map(f32, (w_cls1, b_cls1, w_cls2, b_cls2))

    # host-side constants: loc grid -> loc MLP (batch-independent, replicated)
    ys = np.linspace(-10.0, 10.0, H, dtype=np.float64)
    xs = np.linspace(-10.0, 10.0, W, dtype=np.float64)
    gy, gx = np.meshgrid(ys, xs, indexing="ij")
    locs = np.stack([gy.ravel(), gx.ravel()], -1).astype(np.float32)       # [N, 2]
    em_loc = np.maximum(locs @ w_loc1 + b_loc1, 0.0) @ w_loc2 + b_loc2      # [N, 64]
    emlT = np.ascontiguousarray(em_loc.T).astype(npbf16)                    # [64, N]

    # fold the (linear) em_obs layer into the obs-loc layer
    Wf = w_obs2 @ w_ol1[:EM]                       # [128, 128]
    bfv = b_obs2 @ w_ol1[:EM] + b_ol1              # [128]
    Wl = w_ol1[EM:]                                # [64, 128]

    x2d = images.reshape(B * C, N)

    w1p = np.zeros((35, HID), npbf16)
    w1p[0:C] = w_obs1.astype(npbf16)
    w1p[32:32 + C] = w_obs1.astype(npbf16)

    wdict = {
        "w1": w1p,
        "wf": Wf.astype(npbf16),
        "wl": np.concatenate([Wl, Wl], axis=0).astype(npbf16),
        "w4": w_ol2.astype(npbf16),
        "b1": np.ascontiguousarray(b_obs1[:, None]),
        "bf": np.ascontiguousarray(bfv[:, None]),
        "b4": np.ascontiguousarray(np.concatenate([b_ol2, b_ol2])[:, None]),
    }
    in_maps = []
    for k in range(NCORES):
        sl = slice(k * NTOK, (k + 1) * NTOK)
        m = dict(wdict)
        m["x"] = _pack_x(x2d[:, sl])
        m["eml"] = np.ascontiguousarray(emlT[:, sl])
        in_maps.append(m)

    nc = _get_built()
    global _LAST_IN_MAPS
    _LAST_IN_MAPS = in_maps
    res = run_bass_kernel_spmd(nc, in_maps, list(range(NCORES)))

    em_set = np.zeros((B, EM), np.float32)
    for k in range(NCORES):
        a = np.asarray(res.results[k]["acc"], np.float32)   # [128, ACC_COLS]
        q = a[:, :B * NQUAD].reshape(HID, B, NQUAD).sum(axis=2)  # [128, B]
        em_set += (q[:EM] + q[EM:]).T                       # [B, 64]
        t = a[:, B * NQUAD:]                                # [128, B//2]
        em_set[0::2] += t[:EM].T                            # even samples (top)
        em_set[1::2] += t[EM:].T                            # odd samples (bottom)

    logits = np.maximum(em_set @ w_cls1 + b_cls1, 0.0) @ w_cls2 + b_cls2
    return logits.astype(np.float32)

